# revision 21
# baseline (speedup 1.0000x reference)
"""DiffMLAAttention Trainium2 kernel, tensor-parallel over heads across 8 NeuronCores.

Per-core work (core c): 2 KV heads {2c, 2c+1}, 4 Q heads {4c..4c+3}.
Host folds the low-rank projections into effective weights:
    W_eff_q = W_DQ @ W_UQ,  W_eff_k = W_DKV @ W_UK,  W_eff_v = W_DKV @ W_UV
so the device computes q/k/v directly from x (fp32 fold on host, bf16 on
device).  Weights are column-sharded by head; W_out row-sharded; the host sums
the 8 partial outputs (the row-parallel all-reduce).

Device layout: q^T/k^T produced in [dh, L] layout straight from the matmuls
(no activation transposes); combined probabilities are transposed with the
DMA XBAR (bf16), not the PE.

dtype scheme (validated ~6e-3 rel err vs fp32 reference):
 - x, W_eff_*, W_lam, cos/sin, scores, probs, v, y-partials in bf16
 - W_out in float32r; all accumulation in fp32 PSUM
"""
import math

import numpy as np
import ml_dtypes

import concourse.bass as bass
import concourse.mybir as mybir
import concourse.tile as tile
from concourse import bacc
from concourse.masks import make_causal_mask

P = 128
L = 2048
D = 2048
DC = 512
DCQ = 1024
DH = 128
NH = 16
NCORES = 8
HL = NH // NCORES        # 2 local kv heads
QL = 2 * HL              # 4 local q heads
LT = L // P              # 16 q tiles
F32 = mybir.dt.float32
F32R = mybir.dt.float32r
BF16 = mybir.dt.bfloat16
SCALE = 1.0 / math.sqrt(DH)

XC = 512                 # L-chunk width in projection phase
SC = 1024                # score chunk width (2 fp32 psum banks)


def build_nc(phases=("a", "c", "wo")):
    nc = bacc.Bacc("TRN2", target_bir_lowering=False)

    xT = nc.dram_tensor("xT", [D, L], BF16, kind="ExternalInput")
    weffq = nc.dram_tensor("weffq", [D, QL * DH], BF16, kind="ExternalInput")
    weffk = nc.dram_tensor("weffk", [D, HL * DH], BF16, kind="ExternalInput")
    weffv = nc.dram_tensor("weffv", [D, HL * DH + HL], BF16, kind="ExternalInput")
    nblam = nc.dram_tensor("nblam", [P, HL], F32, kind="ExternalInput")
    wout = nc.dram_tensor("wout", [HL * DH, D], BF16, kind="ExternalInput")
    cosT = nc.dram_tensor("cosT", [DH, L], BF16, kind="ExternalInput")
    sinTs = nc.dram_tensor("sinTs", [DH, L], BF16, kind="ExternalInput")
    y = nc.dram_tensor("y", [L, D], BF16, kind="ExternalOutput")

    xT_r = xT.rearrange("(ko ki) l -> ki ko l", ki=P)
    NKO = D // P

    with tile.TileContext(nc) as tc:
        with (
            tc.tile_pool(name="const", bufs=1) as constp,
            tc.tile_pool(name="big", bufs=1) as bigp,
            tc.tile_pool(name="wa", bufs=1) as wa,
            tc.tile_pool(name="xa", bufs=2) as xa,
            tc.tile_pool(name="tmpa", bufs=2) as tmpa,
            tc.tile_pool(name="wo", bufs=1) as wop,
            tc.tile_pool(name="pp", bufs=3) as pp,
            tc.tile_pool(name="ptp", bufs=2) as ptp,
            tc.tile_pool(name="atp", bufs=2) as atp,
            tc.tile_pool(name="outp", bufs=2) as outp,
            tc.tile_pool(name="smp", bufs=2) as smp,
            tc.tile_pool(name="psum", bufs=2, space="PSUM") as psum,
        ):
            # ---- weight / const DMAs (scalar HWDGE queue, compute order) ----
            cos_sb0 = constp.tile([DH, L], BF16, name="cos_sb")
            nc.scalar.dma_start(cos_sb0[:], cosT[:])
            sin_sb0 = constp.tile([DH, L], BF16, name="sin_sb")
            nc.scalar.dma_start(sin_sb0[:], sinTs[:])
            nblam_sb0 = constp.tile([P, HL], F32, name="nblam_sb")
            nc.scalar.dma_start(nblam_sb0[:], nblam[:])
            weffk_sb = wa.tile([P, NKO, HL * DH], BF16)
            nc.scalar.dma_start(
                weffk_sb[:], weffk.rearrange("(ko ki) m -> ki ko m", ki=P))
            weffv_sb = wa.tile([P, NKO, HL * DH + HL], BF16)
            nc.scalar.dma_start(
                weffv_sb[:], weffv.rearrange("(ko ki) m -> ki ko m", ki=P))
            weffq_sb = wa.tile([P, NKO, QL * DH], BF16)
            nc.scalar.dma_start(
                weffq_sb[:], weffq.rearrange("(ko ki) m -> ki ko m", ki=P))
            cos_sb, sin_sb, nblam_sb = cos_sb0, sin_sb0, nblam_sb0
            cmask = constp.tile([P, P], F32)
            make_causal_mask(nc, cmask[:], mask_val=-1e9)
            wout_sb = wop.tile([P, HL, D], BF16)
            wout_r = wout.rearrange("(ho ki) n -> ki ho n", ki=P)
            nc.scalar.dma_start(wout_sb[:], wout_r[:])

            qT_sb = bigp.tile([P, QL, L], BF16)       # roped q^T  [dh, qh, l]
            kT_sb = bigp.tile([P, HL, L], BF16)       # roped k^T  [dh, h, l]
            v_sb = bigp.tile([P, LT, HL * DH], BF16)  # v          [l%P, lt, h*dh]
            lam_sb = bigp.tile([P, LT, HL], F32)      # sigmoid lambda

            def rope_evict(ps, out_ap, sl):
                """out = ps * cos[:, sl] + shift(ps) * sinTs[:, sl]; out bf16."""
                w = sl.stop - sl.start
                rot = tmpa.tile([P, XC], BF16, tag="rot")
                t2 = tmpa.tile([P, XC], BF16, tag="t2")
                half = DH // 2
                nc.vector.tensor_copy(rot[0:half, :w], ps[half:DH, :])
                nc.vector.tensor_copy(rot[half:DH, :w], ps[0:half, :])
                nc.vector.tensor_mul(rot[:, :w], rot[:, :w], sin_sb[:, sl])
                nc.vector.tensor_mul(t2[:, :w], ps, cos_sb[:, sl])
                nc.vector.tensor_add(out_ap, t2[:, :w], rot[:, :w])

            # ---- Phase A chunk body: x^T chunk -> q^T, k^T, v, lambda ----
            def a_chunk(ncr):
                sl = slice(ncr * XC, (ncr + 1) * XC)
                xt = xa.tile([P, NKO, XC], BF16, tag="xa")
                for kg in range(4):  # split for early compute start
                    nc.sync.dma_start(
                        xt[:, kg * 4:(kg + 1) * 4, :],
                        xT_r[:, kg * 4:(kg + 1) * 4, sl],
                    )
                for h in range(HL):
                    psk = psum.tile([P, XC], F32, tag="aqk", bufs=2)
                    for ko in range(NKO):
                        nc.tensor.matmul(
                            psk[:],
                            weffk_sb[:, ko, h * P:(h + 1) * P],
                            xt[:, ko, :],
                            start=(ko == 0),
                            stop=(ko == NKO - 1),
                        )
                    rope_evict(psk[:], kT_sb[:, h, sl], sl)
                for ls in range(XC // P):
                    lt_idx = ncr * (XC // P) + ls
                    psv = psum.tile([P, 512], F32, tag="sm", bufs=2)
                    for ko in range(NKO):
                        nc.tensor.matmul(
                            psv[:, :HL * DH + HL],
                            xt[:, ko, ls * P:(ls + 1) * P],
                            weffv_sb[:, ko, :],
                            start=(ko == 0),
                            stop=(ko == NKO - 1),
                        )
                    nc.vector.tensor_copy(v_sb[:, lt_idx, :], psv[:, :HL * DH])
                    # lambda = sigmoid(v-chain cols [256:258] + blam)
                    e = tmpa.tile([P, HL], F32, tag="sig")
                    for hh in range(HL):
                        nc.scalar.activation(
                            e[:, hh:hh + 1],
                            psv[:, HL * DH + hh:HL * DH + hh + 1],
                            mybir.ActivationFunctionType.Exp,
                            scale=-1.0,
                            bias=nblam_sb[:, hh:hh + 1],
                        )
                    nc.vector.tensor_scalar_add(e[:], e[:], 1.0)
                    nc.vector.reciprocal(lam_sb[:, lt_idx, :], e[:])
                for qh in range(QL):
                    psq = psum.tile([P, XC], F32, tag="aqk", bufs=2)
                    for ko in range(NKO):
                        nc.tensor.matmul(
                            psq[:],
                            weffq_sb[:, ko, qh * P:(qh + 1) * P],
                            xt[:, ko, :],
                            start=(ko == 0),
                            stop=(ko == NKO - 1),
                        )
                    rope_evict(psq[:], qT_sb[:, qh, sl], sl)

            # ---------------- Phase C j-block: attention + W_out ----------------
            def c_block(j):
                qa, qb = 2 * j, 2 * j + 1
                attnT = atp.tile([P, HL, 2 * P], BF16, tag="at")
                for h in range(HL):
                    ptiles = {}
                    invs = {}
                    for p_ in range(2):
                        qh = 2 * h + p_
                        for qi, qt in enumerate((qa, qb)):
                            lk = (qt + 1) * P
                            nck = (lk + SC - 1) // SC
                            ptile = pp.tile([P, L], BF16, tag=f"P{p_}{qi}")
                            sums = smp.tile([P, 2], F32, tag=f"s{p_}{qi}")
                            for ck in range(nck):
                                cw = min(SC, lk - ck * SC)
                                sps = psum.tile([P, SC], F32, tag="sps", bufs=2)
                                for sb0 in range(0, cw, 512):
                                    sw = min(512, cw - sb0)
                                    nc.tensor.matmul(
                                        sps[:, sb0:sb0 + sw],
                                        qT_sb[:, qh, qt * P:(qt + 1) * P],
                                        kT_sb[:, h, ck * SC + sb0:ck * SC + sb0 + sw],
                                        start=True,
                                        stop=True,
                                    )
                                if ck * SC <= qt * P < ck * SC + cw:
                                    off = qt * P - ck * SC
                                    nc.vector.tensor_add(
                                        sps[:, off:off + P],
                                        sps[:, off:off + P],
                                        cmask[:],
                                    )
                                nc.scalar.activation(
                                    ptile[:, ck * SC:ck * SC + cw],
                                    sps[:, :cw],
                                    mybir.ActivationFunctionType.Exp,
                                    scale=SCALE,
                                    accum_out=sums[:, ck:ck + 1],
                                )
                            if nck > 1:
                                r = smp.tile([P, 1], F32, tag=f"r{p_}{qi}")
                                nc.vector.reduce_sum(
                                    r[:], sums[:, :nck], axis=mybir.AxisListType.X
                                )
                            else:
                                r = sums[:, 0:1]
                            inv = smp.tile([P, 1], F32, tag=f"i{p_}{qi}")
                            nc.vector.reciprocal(inv[:], r[:])
                            ptiles[(p_, qi)] = ptile
                            invs[(p_, qi)] = inv
                    # combined probs:  Pc = P0*inv1 - P1*(lam*inv2)
                    for qi, qt in enumerate((qa, qb)):
                        lk = (qt + 1) * P
                        s2 = smp.tile([P, 1], F32, tag=f"l{qi}")
                        nc.vector.tensor_mul(
                            s2[:], invs[(1, qi)][:], lam_sb[:, qt, h:h + 1]
                        )
                        p0, p1 = ptiles[(0, qi)], ptiles[(1, qi)]
                        nc.vector.tensor_scalar_mul(p1[:, :lk], p1[:, :lk], s2[:])
                        nc.vector.scalar_tensor_tensor(
                            p0[:, :lk],
                            p0[:, :lk],
                            invs[(0, qi)][:],
                            p1[:, :lk],
                            op0=mybir.AluOpType.mult,
                            op1=mybir.AluOpType.subtract,
                        )
                    # transpose combined probs -> PT [lk-part, kc, 2*P] (DMA XBAR)
                    pt = ptp.tile([P, LT, 2 * P], BF16, tag="pt")
                    for qi, qt in enumerate((qa, qb)):
                        for c0 in range(0, qt + 1, 8):
                            c1 = min(c0 + 8, qt + 1)
                            nc.sync.dma_start_transpose(
                                pt[:, c0:c1, qi * P:(qi + 1) * P],
                                ptiles[(0, qi)][:, c0 * P:c1 * P],
                            )
                    # PV
                    pv = psum.tile([P, 512], F32, tag="sm", bufs=2)
                    for kc in range(qa + 1):
                        nc.tensor.matmul(
                            pv[:, :2 * P],
                            v_sb[:, kc, h * DH:(h + 1) * DH],
                            pt[:, kc, :],
                            start=(kc == 0),
                            stop=False,
                        )
                    nc.tensor.matmul(
                        pv[:, P:2 * P],
                        v_sb[:, qb, h * DH:(h + 1) * DH],
                        pt[:, qb, P:2 * P],
                        start=False,
                        stop=True,
                    )
                    nc.vector.tensor_copy(attnT[:, h, :], pv[:, :2 * P])
                # W_out: y[qt] += attn_comb @ wout_local
                for qi, qt in enumerate((qa, qb) if "wo" in phases else ()):
                    osb = outp.tile([P, D], BF16, tag="osb")
                    for nb in range(D // 512):
                        po = psum.tile([P, 512], F32, tag="sm", bufs=2)
                        for h in range(HL):
                            nc.tensor.matmul(
                                po[:],
                                attnT[:, h, qi * P:(qi + 1) * P],
                                wout_sb[:, h, nb * 512:(nb + 1) * 512],
                                start=(h == 0),
                                stop=(h == HL - 1),
                            )
                        if nb % 2 == 0:
                            nc.vector.tensor_copy(
                                osb[:, nb * 512:(nb + 1) * 512], po[:])
                        else:
                            nc.scalar.copy(
                                osb[:, nb * 512:(nb + 1) * 512], po[:])
                    nc.scalar.dma_start(y[qt * P:(qt + 1) * P, :], osb[:])

            # ---- interleaved emission: chunk k, then the j-blocks it unblocks ----
            do_a = "a" in phases
            do_c = "c" in phases
            for ncr in range(L // XC):
                if do_a:
                    a_chunk(ncr)
                if do_c:
                    for j in (2 * ncr, 2 * ncr + 1):
                        c_block(j)

    nc.compile()
    return nc


_NC = None


def _get_nc():
    global _NC
    if _NC is None:
        _NC = build_nc()
    return _NC


def _rope_tables():
    inv_freq = 1.0 / (10000.0 ** (np.arange(0, DH, 2, dtype=np.float32) / DH))
    t = np.arange(L, dtype=np.float32)
    freqs = np.outer(t, inv_freq)                    # [L, DH/2]
    emb = np.concatenate([freqs, freqs], axis=-1)    # [L, DH]
    cos = np.cos(emb).astype(np.float32)
    sin = np.sin(emb).astype(np.float32)
    sign = np.where(np.arange(DH) < DH // 2, -1.0, 1.0).astype(np.float32)
    cosT = np.ascontiguousarray(cos.T)               # [DH, L]
    sinTs = np.ascontiguousarray(sin.T * sign[:, None])
    return cosT, sinTs


def _bf(a):
    return np.ascontiguousarray(np.asarray(a, dtype=np.float32)).astype(
        ml_dtypes.bfloat16
    )


def prepare_in_maps(x, W_DKV, W_UK, W_UV, W_DQ, W_UQ, W_lam, b_lam, W_out):
    x = np.asarray(x, dtype=np.float32)
    W_DKV = np.asarray(W_DKV, dtype=np.float32)
    W_UK = np.asarray(W_UK, dtype=np.float32)
    W_UV = np.asarray(W_UV, dtype=np.float32)
    W_DQ = np.asarray(W_DQ, dtype=np.float32)
    W_UQ = np.asarray(W_UQ, dtype=np.float32)
    W_lam = np.asarray(W_lam, dtype=np.float32)
    b_lam = np.asarray(b_lam, dtype=np.float32)
    W_out = np.asarray(W_out, dtype=np.float32)

    # Host-side low-rank fold (fp32 BLAS)
    Weffq = W_DQ @ W_UQ                              # [D, 2*NH*DH]
    Weffk = W_DKV @ W_UK                             # [D, NH*DH]
    Weffv = W_DKV @ W_UV

    xT_bf = _bf(x[0].T)                              # [D, L] bf16
    cosT, sinTs = _rope_tables()
    cosT_bf, sinTs_bf = _bf(cosT), _bf(sinTs)

    in_maps = []
    for c in range(NCORES):
        nblam = np.ascontiguousarray(
            np.broadcast_to(-b_lam[2 * c:2 * c + 2][None, :], (P, HL))
        ).astype(np.float32)
        in_maps.append({
            "xT": xT_bf,
            "weffq": _bf(Weffq[:, 4 * c * 128:(4 * c + 4) * 128]),
            "weffk": _bf(Weffk[:, c * 256:(c + 1) * 256]),
            "weffv": _bf(np.concatenate(
                [Weffv[:, c * 256:(c + 1) * 256],
                 W_lam[:, 2 * c:2 * c + 2]], axis=1)),
            "nblam": nblam,
            "wout": _bf(W_out[c * 256:(c + 1) * 256, :]),
            "cosT": cosT_bf,
            "sinTs": sinTs_bf,
        })
    return in_maps


def kernel(x, W_DKV, W_UK, W_UV, W_DQ, W_UQ, W_lam, b_lam, W_out):
    in_maps = prepare_in_maps(
        x, W_DKV, W_UK, W_UV, W_DQ, W_UQ, W_lam, b_lam, W_out)

    from concourse.bass_utils import run_bass_kernel_spmd
    nc = _get_nc()
    res = run_bass_kernel_spmd(nc, in_maps, core_ids=list(range(NCORES)))
    y = np.zeros((L, D), dtype=np.float32)
    for c in range(NCORES):
        y += res.results[c]["y"].astype(np.float32)
    return y.reshape(1, L, D)
